# revision 15
# baseline (speedup 1.0000x reference)
"""AttentivePool Trainium2 kernel.

Reference computation (per batch sample b):
    m[c, w]   = mean_h x[b, c, h, w]                      # H-mean pool
    s[c', w]  = tanh(sum_c W[c, c'] m[c, w] + bias[c'])   # additive attention
    a[w]      = sum_c' s[c', w] proj[c']
    p[w]      = softmax_w(a)                              # over W
    out[b, c] = sum_w p[w] m[c, w]

Strategy: pure data-parallel over B across 8 cores (2 samples/core).
x is 1 GiB; everything else is tiny -> memory-bound on streaming x.

DMA layout is the whole game: loading [C, H, wt] w-chunks (as a strided
reduce would want) gives only wt*4-byte contiguous HBM segments — 1 KiB
at wt=256 — and measures ~260 GB/s/core.  Loading h-slices x[b, :, h, :]
as [C, W] tiles gives 16 KiB contiguous segments per partition and
measures ~354 GB/s/core, at the ~358 GB/s HBM-per-NeuronCore limit
(716 GB/s/stack shared by 2 cores).  So: stream 32 h-slices per sample
(2 MiB DMAs alternating between the SP and ACT HWDGE rings) and form
the H-sum with 31 in-place DVE tensor_adds (fp32 tensor_tensor runs at
1 elem/cycle/lane -> 4.4 us per add, under the 5.9 us DMA inter-arrival,
so DVE hides completely under the DMA stream).

The attention tail runs per 512-wide chunk (one PSUM bank each): PE
matmul (W/H)^T m -> ACT tanh+bias -> PE proj matmul -> ACT exp (with
accum_out emitting the chunk-sum for the softmax denominator) -> PE
ones-broadcast -> DVE mul+reduce for the weighted partial sums.  Each
chunk tail of sample b is split into 4 stages advanced one per add-slot
of sample b+1's stream, so every ACT op's input is >=1 slot old and the
x-load dma_starts issued from the ACT ring never wait behind a stalled
activation (and the in-order DVE queue never stalls on the tail's
cross-engine latency).  The last sample's final h-slice arrives as 4
quarter-DMAs with per-quarter chunk-adds + inline tails, shrinking the
end-of-kernel drain.  Softmax skips the max-subtraction: |a| <=
sum|proj| = 12.8 hard bound, exp is safe in f32.

Scaling: the adds compute H*mean; the host folds 1/H into weight_W and
into the broadcast ones-vector so all downstream values come out exact
(1/H is a power of two).
"""

import contextlib

import numpy as np

import concourse.bacc as bacc
import concourse.tile as tile
from concourse import mybir
from concourse.bass_utils import run_bass_kernel_spmd

B, C, H, W = 16, 128, 32, 4096
N_CORES = 8
BL = B // N_CORES  # batch samples per core
WT = 512           # attention-tail chunk width (one 2 KiB PSUM bank)
F32 = mybir.dt.float32


def build_bass(bl=BL, w=W, wt=WT, reps=1, loop_reps=1, xbufs=5, dual_q=True,
               dma_scratch=16384, tbufs=2, pbufs=(2, 2, 2), no_tail=False,
               staggered=False, rings=("sync", "scalar")):
    nch = w // wt
    # Bacc (not plain Bass): its compile() runs generate_event_semaphores,
    # which spills >1-wait sync conditions into EventSemaphore instructions
    # (the TRN2 ISA allows a single wait slot per instruction).
    nc = bacc.Bacc(trn_type="TRN2", dynamic_dma_scratch_size=dma_scratch)

    x = nc.dram_tensor("x", [bl, C, H, w], F32, kind="ExternalInput")
    # All small parameters packed into one tensor: a single DMA means every
    # PE matmul depends on a single weight semaphore (the Matmult/LdWeights
    # sync slot only fits ONE wait, so fan-in must stay at 1).
    # cols 0:128 = weight_W/H, 128:256 = eye(C), 256 = proj, 257 = bias,
    # row 0 of cols 258:386 = 1/H (ones row for broadcasts).
    wpack = nc.dram_tensor("wpack", [C, 386], F32, kind="ExternalInput")
    out = nc.dram_tensor("out", [bl, C], F32, kind="ExternalOutput")

    with tile.TileContext(nc) as tc:
        with (
            tc.tile_pool(name="singles", bufs=1) as singles,
            tc.tile_pool(name="xp", bufs=xbufs) as xpool,
            tc.tile_pool(name="mp", bufs=1) as mpool,
            tc.tile_pool(name="sqp", bufs=tbufs) as sqpool,
            tc.tile_pool(name="ep", bufs=tbufs) as epool,
            tc.tile_pool(name="accp", bufs=1) as accp,
            tc.tile_pool(name="psp", bufs=pbufs[0], space="PSUM") as psp,
            tc.tile_pool(name="pbp", bufs=pbufs[1], space="PSUM") as pbp,
            tc.tile_pool(name="psmall", bufs=pbufs[2], space="PSUM") as psmall,
        ):
            # wpack rides the ACT ring so x h-slice 0 (SP ring) starts at t=0.
            sb_w = singles.tile([C, 386], F32, tag="wpack")
            nc.scalar.dma_start(out=sb_w, in_=wpack[:, :])
            sb_ww = sb_w[:, 0:C]
            sb_ident = sb_w[:, C:2 * C]
            sb_proj = sb_w[:, 2 * C:2 * C + 1]
            sb_bias = sb_w[:, 2 * C + 1:2 * C + 2]
            sb_ones = sb_w[0:1, 2 * C + 2:3 * C + 2]

            # Dummy matmul so PE observes the wpack DMA semaphore before the
            # main loop; later matmuls then only wait on their data operand.
            scr = psmall.tile([1, 1], F32, tag="small")
            nc.tensor.matmul(scr, sb_proj, sb_bias, start=True, stop=True)

            # Per-(b, chunk) accumulators, each slot written exactly once.
            partials = accp.tile([C, bl, nch], F32, tag="partials")
            dparts = accp.tile([1, bl, nch], F32, tag="dparts")
            if no_tail:
                nc.vector.memset(partials, 0.0)
                nc.vector.memset(dparts, 0.0)

            # The attention tail for one chunk, split into 4 stages so it
            # can be advanced one stage per add-slot of the NEXT sample's
            # stream: by the time each ACT op (tanh/exp) reaches the front
            # of the ACT queue its input is >=1 slot old, so the x-load
            # dma_starts issued from ACT never wait behind a stalled
            # activation.
            pending = []  # queued (depth, stage_fn, args) tail work

            def s_tanh(b, k, mt):
                # squish = tanh((W/H)^T (H*mean) + bias) on chunk k of m_b
                pst = psp.tile([C, wt], F32, tag="ps", name="pst")
                nc.tensor.matmul(pst, sb_ww, mt, start=True, stop=True)
                sqt = sqpool.tile([C, wt], F32, tag="sq", name="sqt")
                nc.scalar.activation(
                    out=sqt, in_=pst,
                    func=mybir.ActivationFunctionType.Tanh,
                    bias=sb_bias, scale=1.0,
                )
                pending.append((1, s_proj, (b, k, mt, sqt)))

            def s_proj(b, k, mt, sqt):
                # attn chunk = proj^T squish  -> [1, wt]  (PE only)
                pat = psmall.tile([1, wt], F32, tag="small", name="pat")
                nc.tensor.matmul(pat, sb_proj, sqt, start=True, stop=True)
                pending.append((2, s_exp, (b, k, mt, pat)))

            def s_exp(b, k, mt, pat):
                # exp (softmax numerator); accum_out = chunk sum for denom
                et = epool.tile([1, wt], F32, tag="et", name="et")
                nc.scalar.activation(
                    out=et, in_=pat,
                    func=mybir.ActivationFunctionType.Exp,
                    accum_out=dparts[0:1, b, k:k + 1],
                )
                pending.append((3, s_wsum, (b, k, mt, et)))

            def s_wsum(b, k, mt, et):
                # broadcast exp/H to all partitions, then
                # (exp/H)*(H*mean) summed over w -> partials[:, b, k].
                # (tensor_tensor_reduce hard-faults TRN2, so separate
                # mul — in-place over the dead m chunk — plus reduce.)
                pbt = pbp.tile([C, wt], F32, tag="pb", name="pbt")
                nc.tensor.matmul(pbt, sb_ones, et, start=True, stop=True)
                nc.vector.tensor_mul(out=mt, in0=pbt, in1=mt)
                nc.vector.reduce_sum(
                    out=partials[:, b, k:k + 1], in_=mt,
                    axis=mybir.AxisListType.X,
                )

            def advance(n=1):
                # Drain-first: advance the deepest-stage item so each
                # chunk's tail runs inline-sequential spread across slots
                # (1 live sqt/pat/et tile at a time -> bufs=2 pools).
                for _ in range(n):
                    if pending:
                        idx = max(range(len(pending)),
                                  key=lambda i: pending[i][0])
                        _, fn, args = pending.pop(idx)
                        fn(*args)

            def tail_head(b, k, mt):
                pst = psp.tile([C, wt], F32, tag="ps", name="pst")
                nc.tensor.matmul(pst, sb_ww, mt, start=True, stop=True)
                sqt = sqpool.tile([C, wt], F32, tag="sq", name="sqt")
                nc.scalar.activation(
                    out=sqt, in_=pst,
                    func=mybir.ActivationFunctionType.Tanh,
                    bias=sb_bias, scale=1.0,
                )
                pat = psmall.tile([1, wt], F32, tag="small", name="pat")
                nc.tensor.matmul(pat, sb_proj, sqt, start=True, stop=True)
                et = epool.tile([1, wt], F32, tag="et", name="et")
                nc.scalar.activation(
                    out=et, in_=pat,
                    func=mybir.ActivationFunctionType.Exp,
                    accum_out=dparts[0:1, b, k:k + 1],
                )
                return et

            def tail_rest(b, k, mt, et):
                pbt = pbp.tile([C, wt], F32, tag="pb", name="pbt")
                nc.tensor.matmul(pbt, sb_ones, et, start=True, stop=True)
                nc.vector.tensor_mul(out=mt, in0=pbt, in1=mt)
                nc.vector.reduce_sum(
                    out=partials[:, b, k:k + 1], in_=mt,
                    axis=mybir.AxisListType.X,
                )

            def attention_tail(b, k, mt):
                et = tail_head(b, k, mt)
                tail_rest(b, k, mt, et)

            loop_cm = (
                tc.For_i(0, loop_reps, 1, staggered_reset=staggered)
                if loop_reps > 1
                else contextlib.nullcontext()
            )
            ring_engs = [getattr(nc, r) for r in rings]
            with loop_cm:
              for _rep in range(reps):
                ms = []
                nq = 4  # quarter-DMAs for the last sample's final h-slice
                for b in range(bl):
                    m = mpool.tile([C, w], F32, tag=f"m{b}")
                    ms.append(m)
                    last = b == bl - 1
                    # For the last sample, the final h-slice arrives as nq
                    # quarter-DMAs so its chunk-adds (and their tails) can
                    # start as each quarter lands -> short end drain.
                    h_full = H - 1 if last else H
                    for h in range(H):
                        i = b * H + h
                        dma_eng = ring_engs[i % len(ring_engs)] if dual_q \
                            else ring_engs[0]
                        if h == 0:
                            # First h-slice lands directly in the m
                            # accumulator; no copy needed.
                            dma_eng.dma_start(out=m, in_=x[b, :, 0, :])
                            continue
                        if h < h_full:
                            xt = xpool.tile([C, w], F32, tag="xt")
                            dma_eng.dma_start(out=xt, in_=x[b, :, h, :])
                            nc.vector.tensor_add(out=m, in0=m, in1=xt)
                            advance()
                            continue
                        # last sample, h == H-1: quarter-DMAs + chunk adds
                        # + inline tails per chunk.
                        while pending:  # flush older tail stages
                            advance()
                        wq = w // nq
                        for q in range(nq):
                            qs = slice(q * wq, (q + 1) * wq)
                            xt = xpool.tile([C, wq], F32, tag="xq")
                            eng = ring_engs[q % len(ring_engs)] if dual_q \
                                else ring_engs[0]
                            eng.dma_start(out=xt, in_=x[b, :, h, qs])
                            for j in range(wq // wt):
                                k = q * (wq // wt) + j
                                ws = slice(k * wt, (k + 1) * wt)
                                jw = slice(j * wt, (j + 1) * wt)
                                nc.vector.tensor_add(
                                    out=m[:, ws], in0=m[:, ws], in1=xt[:, jw])
                                if not no_tail and not (q == nq - 1
                                                        and j == wq // wt - 1):
                                    attention_tail(b, k, m[:, ws])
                    if not no_tail and not last:
                        for k in range(nch):
                            pending.append(
                                (0, s_tanh,
                                 (b, k, ms[b][:, k * wt:(k + 1) * wt]))
                            )
                # Drain: the last chunk's tail is split around the softmax-
                # denominator chain — after its exp, dparts is complete, so
                # the denominator math runs in parallel with the final
                # weighted-sum ops instead of serially after them.
                pdb = psmall.tile([C, bl], F32, tag="pdb")
                if no_tail:
                    drow = accp.tile([1, bl], F32, tag="drow")
                    nc.vector.memset(drow, 1.0)
                    nc.tensor.matmul(pdb, sb_ones, drow, start=True, stop=True)
                else:
                    lb, lk = bl - 1, nch - 1
                    lmt = ms[lb][:, lk * wt:(lk + 1) * wt]
                    let = tail_head(lb, lk, lmt)
                    drow = accp.tile([1, bl], F32, tag="drow")
                    nc.vector.reduce_sum(
                        out=drow, in_=dparts, axis=mybir.AxisListType.X)
                    nc.scalar.mul(out=drow, in_=drow, mul=1.0 / H)
                    nc.vector.reciprocal(out=drow, in_=drow)  # H/denom
                    # (1/H) ones^T @ (H/denom) = 1/denom bcast to all parts
                    nc.tensor.matmul(pdb, sb_ones, drow, start=True, stop=True)
                    tail_rest(lb, lk, lmt, let)

            rescol = accp.tile([C, bl], F32, tag="rescol")
            nc.vector.reduce_sum(out=rescol, in_=partials, axis=mybir.AxisListType.X)
            resn = accp.tile([C, bl], F32, tag="resn")
            nc.vector.tensor_mul(out=resn, in0=rescol, in1=pdb)

            # out[b, c] = resn[c, b]: transpose via matmul with identity.
            pt = psmall.tile([bl, C], F32, tag="small")
            nc.tensor.matmul(pt, resn, sb_ident, start=True, stop=True)
            out_sb = accp.tile([bl, C], F32, tag="out_sb")
            nc.vector.tensor_copy(out=out_sb, in_=pt)
            nc.sync.dma_start(out=out[:, :], in_=out_sb)

    nc.compile()
    return nc


def make_in_maps(x, weight_W, weight_proj, bias, bl=BL, n_cores=N_CORES):
    x = np.ascontiguousarray(np.asarray(x, dtype=np.float32))
    wpack = np.zeros((C, 386), dtype=np.float32)
    wpack[:, 0:C] = np.asarray(weight_W, dtype=np.float32) / np.float32(H)
    wpack[:, C:2 * C] = np.eye(C, dtype=np.float32)
    wpack[:, 2 * C:2 * C + 1] = np.asarray(weight_proj, dtype=np.float32)
    wpack[:, 2 * C + 1:2 * C + 2] = np.asarray(bias, dtype=np.float32)
    wpack[0, 2 * C + 2:3 * C + 2] = 1.0 / np.float32(H)
    return [
        {
            "x": np.ascontiguousarray(x[i * bl:(i + 1) * bl]),
            "wpack": wpack,
        }
        for i in range(n_cores)
    ]


_NC_CACHE = {}


def kernel(x, weight_W, weight_proj, bias, **run_kwargs):
    if "nc" not in _NC_CACHE:
        _NC_CACHE["nc"] = build_bass()
    nc = _NC_CACHE["nc"]
    in_maps = make_in_maps(x, weight_W, weight_proj, bias)
    res = None
    for attempt in range(3):
        try:
            res = run_bass_kernel_spmd(
                nc, in_maps, core_ids=list(range(N_CORES)), **run_kwargs)
            break
        except Exception:
            # Transient NRT/device hiccups recover on retry; re-raise if not.
            if attempt == 2:
                raise
    out = np.concatenate([r["out"] for r in res.results], axis=0)
    if run_kwargs:
        kernel.last_results = res
    return out
